# revision 60
# baseline (speedup 1.0000x reference)
"""MoE routing kernel for Trainium2, expert-parallel across 8 NeuronCores.

Strategy (mirrors the module's parallel_forward_once path):
  - Router (softmax -> top-2 -> capacity-limited dispatch indices) is computed
    on host with jax-on-CPU, replicating the reference bit-exactly (it is
    ~34 MFLOP, negligible).
  - Tokens are gathered per (k, expert) into capacity slots on host (the
    "all-to-all"), shipped transposed as [hs, 2*cap] per expert in bf16.
  - Each of the 8 cores runs one expert's FFN: y^T = w2^T @ gelu(w1^T @ x^T)
    with bf16 matmuls on the PE (f32 PSUM accumulate) and tanh-gelu on ACT.
  - Host scatters the per-expert outputs back with the top-k weights.

Weights/activations are pre-packed on host into PE-tile-major layouts so
every DMA moves contiguous 2KB per-partition lines, and are cast to bf16
(rel err ~3e-3 vs the 2e-2 tolerance; DMA bytes halve and matmuls run at
1.0 cycles/row, same as fp32r at this tile width but with no minimum
free-dim constraint).

Schedule notes (targets the TimelineSim cost model, ~226.6us vs a
~225.5us structural floor of 3.7us startup latency + 218.1us of matmul
columns + 3.5us store-drain tail; the residual is a cluster of ~8
balanced supply-latency critical paths):
  - Junk warm-up matmuls on a zeroed tile start the PE p-state ramp
    ~3us before the first real operand lands, so real matmuls run at
    2.4GHz from (nearly) the first one.
  - xe streams on SP/HWDGE; the first-consumed w1 half-tiles stream on
    gpsimd/SWDGE in parallel. Super-group 0 (mt0/mt1, kt-major,
    8 psum-bank groups) consumes chunks in arrival order.
  - A Pool memset spacer keeps the streamed w1 tiles' DMA-engine
    requests behind the last xe chunk's (the DMA engines serve one
    transfer at a time, in request order), and the critical-path-peak
    xe chunk is split into halves so its second half carries less
    forced-serial work.
  - Phase 2 contracts all 32 k2 tiles into one psum group per output
    tile and stores via DVE-copy + SP DMA; the last group is split
    256/128/128 so the final copy+store chain is short.

Problem shape (hardcoded): x [2048, 2, 1024], router_w [1024, 8],
w1 [8, 1024, 4096], w2 [8, 4096, 1024], bias [1, 1, 1024].
"""

import os

import numpy as np

NUM_EXPERTS = 8
TOP_K = 2
HS = 1024
FFN = 4096
SL, BS = 2048, 2
TOKENS = SL * BS  # 4096
CAP = TOKENS // NUM_EXPERTS  # 512
COLS = TOP_K * CAP  # 1024 dispatch slots per expert (both k passes)

P = 128
KT = HS // P  # 8 contraction tiles (phase 1)
MT = FFN // P  # 32 ffn row tiles (phase 1 out / phase 2 contraction)
M2 = HS // P  # 8 output row tiles (phase 2)
NT = 2
NTW = COLS // NT  # 512

_CACHE = {}
_LAST_RESULTS = None  # test harness introspection


def _build_nc(n_junk=26, split_kt=(5,), out_bufs=3, w2_bufs=3, xe_fp8=False):
    import concourse.bacc as bacc
    import concourse.mybir as mybir
    import concourse.tile as tile

    dt = mybir.dt
    f32 = dt.float32
    bf16 = dt.bfloat16

    nc = bacc.Bacc(
        "TRN2", target_bir_lowering=False, debug=False, num_devices=NUM_EXPERTS
    )

    # Host-packed layouts (see kernel()):
    #   xeT [kt, p, cols]           = x^T tiles, bf16
    #   w1p [mt, p, kt, c]          = w1[kt*128+p, mt*128+c], bf16
    #   w2p [m2, p, k2, c]          = w2[k2*128+p, m2*128+c], bf16
    #   yT  [m2, p, cols]           = y^T tiles, f32
    xedt = dt.float8e3 if xe_fp8 else bf16
    xeT = nc.dram_tensor("xeT", [KT, P, COLS], xedt, kind="ExternalInput")
    w1p = nc.dram_tensor("w1p", [MT, P, KT, P], bf16, kind="ExternalInput")
    w2p = nc.dram_tensor("w2p", [M2, P, MT, P], bf16, kind="ExternalInput")
    yT = nc.dram_tensor("yT", [M2, P, COLS], f32, kind="ExternalOutput")

    gelu = mybir.ActivationFunctionType.Gelu_apprx_tanh

    with tile.TileContext(nc) as tc:
        with (
            tc.tile_pool(name="resident", bufs=1) as resident,
            tc.tile_pool(name="w1pool", bufs=6) as w1pool,
            tc.tile_pool(name="w2pool", bufs=w2_bufs) as w2pool,
            tc.tile_pool(name="outpool", bufs=out_bufs) as outpool,
            tc.tile_pool(name="psum", bufs=8, space="PSUM") as psum_pool,
        ):
            # --- PE warm-up -------------------------------------------------
            # The cost of a matmul depends on how long the PE has been busy
            # (p-state ramp). Junk matmuls issued immediately (zeroed operand,
            # results never read) start the busy clock ~3us before the first
            # real matmul's operands arrive, so real matmuls run at full
            # clock from the start. The memset runs on the otherwise-idle
            # DVE so it doesn't delay the gpsimd DMA stream.
            with tc.high_priority():
                junk = resident.tile([P, P], bf16, tag="junk")
                nc.vector.memset(junk[:], 0)
                jpsum = psum_pool.tile([P, NTW], f32, tag="ps", name="jpsum")
                for _ in range(n_junk):
                    nc.tensor.matmul(
                        jpsum[:, :P], junk[:], junk[:], start=True, stop=True
                    )

            # --- activation tiles (SP/HWDGE stream; xe only) ----------------
            # kt0 is split into nt halves so the first matmul only waits for
            # a 128KB transfer; kt1..7 are whole (728ns each, just above the
            # 625ns/DMA HWDGE cadence).
            KH = KT // 2
            SG0 = 2
            # xe_fp8 (measured WORSE, default off): shipping activations as
            # fp8-e3m4 halves their FIFO time but the ~1us on-chip upcast
            # (DVE/ACT) exceeds the 364ns/chunk transfer saving, re-binding
            # the critical-path cluster (+388ns net; err would be 1.4e-2).
            cast_eng = [0]

            def load_xe(name, dst_slice, width):
                t = resident.tile([P, width], bf16, tag=name, name=name)
                if xe_fp8:
                    t8 = resident.tile([P, width], xedt, tag=name + "_8", name=name + "_8")
                    nc.sync.dma_start(t8[:], dst_slice)
                    if cast_eng[0] % 2 == 0:
                        nc.vector.tensor_copy(t[:], t8[:])
                    else:
                        nc.scalar.activation(
                            t[:], t8[:], mybir.ActivationFunctionType.Copy
                        )
                    cast_eng[0] += 1
                else:
                    nc.sync.dma_start(t[:], dst_slice)
                return t

            xek = [None] * KT  # whole-kt tiles (kt >= 1)
            xe0 = [None, None]  # kt0 nt halves
            xe0[0] = load_xe("xe0n0", xeT.ap()[0, :, 0:NTW], NTW)
            xe0[1] = load_xe("xe0n1", xeT.ap()[0, :, NTW:COLS], NTW)
            # Cluster-peak chunks: halve so the second half's arrival
            # carries ~0.4us less forced-serial work.
            xes = {k: [None, None] for k in split_kt}
            for kt in range(1, KT):
                if kt in xes:
                    for nt in range(NT):
                        xes[kt][nt] = load_xe(
                            f"xe{kt}n{nt}",
                            xeT.ap()[kt, :, nt * NTW : (nt + 1) * NTW],
                            NTW,
                        )
                    continue
                xek[kt] = load_xe(f"xe{kt}", xeT.ap()[kt], COLS)

            def xe_slice(kt, nt):
                if kt == 0:
                    return xe0[nt][:]
                if kt in xes:
                    return xes[kt][nt][:]
                return xek[kt][:, nt * NTW : (nt + 1) * NTW]

            # --- w1 tiles (gpsimd/SWDGE stream, parallel to HWDGE) ----------
            # mt0/mt1 come first as kt halves (the kt-major super-group 0
            # consumes them with ~1us latency); the rest are whole tiles,
            # fully resident so the Tile scheduler never stalls the Pool
            # queue on slot release and the w2 loads stay behind all of w1
            # instead of being hoisted into the startup window.
            t0a = w1pool.tile([P, KH, P], bf16, tag="w1a", bufs=2)
            nc.gpsimd.dma_start(t0a[:], w1p.ap()[0, :, 0:KH, :])
            t1a = w1pool.tile([P, KH, P], bf16, tag="w1a", bufs=2)
            nc.gpsimd.dma_start(t1a[:], w1p.ap()[1, :, 0:KH, :])
            t0b = w1pool.tile([P, KH, P], bf16, tag="w1b", bufs=2)
            nc.gpsimd.dma_start(t0b[:], w1p.ap()[0, :, KH:KT, :])
            t1b = w1pool.tile([P, KH, P], bf16, tag="w1b", bufs=2)
            nc.gpsimd.dma_start(t1b[:], w1p.ap()[1, :, KH:KT, :])

            ta = [t0a, t1a]
            tb = [t0b, t1b]

            def w1_slice_sg0(mt, kt):
                if kt < KH:
                    return ta[mt][:, kt]
                return tb[mt][:, kt - KH]

            # Pool spacer: nudges w1t2's descriptor-gen (and thus its
            # DMA-engine request) past the last xe chunk's, so the xe stream
            # finishes earlier. (Pool is otherwise idle here.)
            pfill = resident.tile([P, 2048], bf16, tag="pfill")
            nc.gpsimd.memset(pfill[:], 0)

            # Streamed w1 tiles for mt2..31 (gpsimd; fully resident).
            w1t = {}
            for mt in range(SG0, MT):
                t = w1pool.tile(
                    [P, KT, P], bf16, tag="w1t", bufs=MT - SG0, name=f"w1t{mt}"
                )
                nc.gpsimd.dma_start(t[:], w1p.ap()[mt])
                w1t[mt] = t

            hT = resident.tile([P, MT, COLS], bf16, tag="hT")

            # --- phase 1: hT = gelu(w1^T @ xeT) -----------------------------
            # Super-group 0 (mt0, mt1): kt-major so each arriving xe chunk /
            # w1 half-tile unlocks work immediately (supply-latency-bound
            # region). Remaining mt: group-major (all data resident by then).
            ps01 = [
                [
                    psum_pool.tile([P, NTW], f32, tag="ps", name=f"ps{mt}{nt}")
                    for nt in range(NT)
                ]
                for mt in range(SG0)
            ]
            for kt in range(KT):
                # For split chunks, consume in arrival order (both nt0
                # matmuls before nt1); otherwise mt-major.
                order = (
                    [(mt, nt) for nt in range(NT) for mt in range(SG0)]
                    if kt in xes
                    else [(mt, nt) for mt in range(SG0) for nt in range(NT)]
                )
                for mt, nt in order:
                    nc.tensor.matmul(
                        ps01[mt][nt][:],
                        w1_slice_sg0(mt, kt),
                        xe_slice(kt, nt),
                        start=(kt == 0),
                        stop=(kt == KT - 1),
                    )
            for mt in range(SG0):
                for nt in range(NT):
                    nc.scalar.activation(
                        hT[:, mt, nt * NTW : (nt + 1) * NTW], ps01[mt][nt][:], gelu
                    )

            for mt in range(SG0, MT):
                for nt in range(NT):
                    ps = psum_pool.tile([P, NTW], f32, tag="ps")
                    for kt in range(KT):
                        nc.tensor.matmul(
                            ps[:],
                            w1t[mt][:, kt],
                            xe_slice(kt, nt),
                            start=(kt == 0),
                            stop=(kt == KT - 1),
                        )
                    nc.scalar.activation(
                        hT[:, mt, nt * NTW : (nt + 1) * NTW], ps[:], gelu
                    )

            # --- phase 2: yT = w2^T @ hT ------------------------------------
            # w2 tiles stream on gpsimd (the Pool stream is idle after w1).
            # Outputs are evicted PSUM->SBUF on the otherwise-idle DVE, then
            # stored from the SP queue. The last group is split into two
            # 256-col groups so the final eviction+store chain is short.
            w2t = {}

            def load_w2(m2):
                t = w2pool.tile([P, MT, P], bf16, tag="w2t")
                nc.gpsimd.dma_start(t[:], w2p.ap()[m2])
                w2t[m2] = t

            load_w2(0)
            load_w2(1)
            for m2 in range(M2):
                if m2 + 2 < M2:
                    load_w2(m2 + 2)
                for nt in range(NT):
                    last = m2 == M2 - 1 and nt == NT - 1
                    # The chronologically-last store's copy+DMA chain is the
                    # program tail; narrow pieces shorten it.
                    widths = (
                        [(0, NTW)]
                        if not last
                        else [
                            (0, NTW // 2),
                            (NTW // 2, 3 * NTW // 4),
                            (3 * NTW // 4, 7 * NTW // 8),
                            (7 * NTW // 8, NTW),
                        ]
                    )
                    for c0, c1 in widths:
                        ps2 = psum_pool.tile([P, c1 - c0], f32, tag="ps")
                        for k2 in range(MT):
                            nc.tensor.matmul(
                                ps2[:],
                                w2t[m2][:, k2],
                                hT[:, k2, nt * NTW + c0 : nt * NTW + c1],
                                start=(k2 == 0),
                                stop=(k2 == MT - 1),
                            )
                        ysb = outpool.tile([P, c1 - c0], f32, tag="ysb", name="ysb")
                        nc.vector.tensor_copy(ysb[:], ps2[:])
                        nc.sync.dma_start(
                            yT.ap()[m2, :, nt * NTW + c0 : nt * NTW + c1], ysb[:]
                        )
    nc.finalize()
    return nc


def _routing(x, router_w):
    """Replicates the reference's routing decisions bit-exactly on jax-CPU.

    Returns (expert_weights [tokens, K] np.f32,
             tok_idx  [K, E, CAP] np.int64 token index per slot,
             valid    [K, E, CAP] np.bool_).
    """
    import jax
    import jax.numpy as jnp

    cpu = jax.devices("cpu")[0]
    with jax.default_device(cpu):
        xf = jnp.asarray(np.asarray(x, dtype=np.float32).reshape(TOKENS, HS))
        rw = jnp.asarray(np.asarray(router_w, dtype=np.float32))
        scores = jax.nn.softmax(xf @ rw, axis=-1)
        expert_weights, top_experts = jax.lax.top_k(scores, TOP_K)

        tok_idx = np.zeros((TOP_K, NUM_EXPERTS, CAP), np.int64)
        valid = np.zeros((TOP_K, NUM_EXPERTS, CAP), np.bool_)
        for k in range(TOP_K):
            te = top_experts[:, k].astype(jnp.int32)
            tpe = jnp.bincount(te, length=NUM_EXPERTS)
            indices = jnp.argsort(te)  # stable sort by expert id
            offsets = jnp.concatenate(
                [jnp.zeros((1,), tpe.dtype), jnp.cumsum(tpe)[:-1]]
            )
            slot = jnp.arange(CAP)
            pos = offsets[:, None] + slot[None, :]
            v = slot[None, :] < tpe[:, None]
            ti = indices[jnp.minimum(pos, TOKENS - 1)]
            tok_idx[k] = np.asarray(ti)
            valid[k] = np.asarray(v)
        ew = np.asarray(expert_weights, dtype=np.float32)
    return ew, tok_idx, valid


def kernel(x, router_w, w1, w2, bias):
    global _LAST_RESULTS
    import ml_dtypes

    from concourse.bass_utils import run_bass_kernel_spmd

    bf16 = ml_dtypes.bfloat16

    x = np.asarray(x, dtype=np.float32)
    router_w = np.asarray(router_w, dtype=np.float32)
    w1 = np.asarray(w1, dtype=np.float32)
    w2 = np.asarray(w2, dtype=np.float32)
    bias = np.asarray(bias, dtype=np.float32)

    ew, tok_idx, valid = _routing(x, router_w)
    xf = x.reshape(TOKENS, HS)

    # Gather tokens into per-expert capacity slots, transposed to [hs, cols],
    # then tile/cast for the device layouts.
    xeT_all = np.zeros((NUM_EXPERTS, HS, COLS), np.float32)
    for k in range(TOP_K):
        xe = xf[tok_idx[k]]  # [E, CAP, HS]
        xe[~valid[k]] = 0.0
        xeT_all[:, :, k * CAP : (k + 1) * CAP] = xe.transpose(0, 2, 1)
    xeT_pack = np.ascontiguousarray(
        xeT_all.reshape(NUM_EXPERTS, KT, P, COLS)
    ).astype(bf16)

    # w1p[mt, p, kt, c] = w1[kt*128+p, mt*128+c]
    w1_pack = np.ascontiguousarray(
        w1.reshape(NUM_EXPERTS, KT, P, MT, P).transpose(0, 3, 2, 1, 4)
    ).astype(bf16)
    # w2p[m2, p, k2, c] = w2[k2*128+p, m2*128+c]
    w2_pack = np.ascontiguousarray(
        w2.reshape(NUM_EXPERTS, MT, P, M2, P).transpose(0, 3, 2, 1, 4)
    ).astype(bf16)

    if "nc" not in _CACHE:
        _CACHE["nc"] = _build_nc()
    nc = _CACHE["nc"]

    in_maps = [
        {"xeT": xeT_pack[e], "w1p": w1_pack[e], "w2p": w2_pack[e]}
        for e in range(NUM_EXPERTS)
    ]
    trace = bool(int(os.environ.get("KERNEL_TRACE", "0")))

    def _run(trace):
        return run_bass_kernel_spmd(
            nc, in_maps, core_ids=list(range(NUM_EXPERTS)), trace=trace
        )

    try:
        res = _run(trace)
    except ModuleNotFoundError:
        # Under axon with BASS_TRACE set but no NTFF hook shipped
        # (stub antenv), the trace path raises on import — run untraced.
        os.environ["BASS_NEVER_TRACE"] = "1"
        try:
            res = _run(False)
        finally:
            del os.environ["BASS_NEVER_TRACE"]
    except Exception as e:
        # A previously-wedged NeuronCore occasionally surfaces as a one-off
        # NRT_EXEC failure on the first execution; a single retry recovers.
        if "UNRECOVERABLE" in str(e) or "UNAVAILABLE" in str(e):
            res = _run(trace)
        else:
            raise
    _LAST_RESULTS = res

    out = np.zeros((TOKENS, HS), np.float32)
    yT_all = np.stack(
        [
            np.asarray(res.results[e]["yT"], dtype=np.float32).reshape(HS, COLS)
            for e in range(NUM_EXPERTS)
        ]
    )
    for k in range(TOP_K):
        yk = yT_all[:, :, k * CAP : (k + 1) * CAP].transpose(0, 2, 1)  # [E, CAP, HS]
        v = valid[k]
        t = tok_idx[k][v]  # unique within one k pass
        out[t] += yk[v] * ew[t, k][:, None]

    return (out.reshape(SL, BS, HS) + bias).astype(np.float32)
